# revision 9
# baseline (speedup 1.0000x reference)
"""Trainium2 Bass kernel for nn_CAModel (neural cellular automata step).

Data-parallel over 8 NeuronCores: 4 images per core.

Per-core layout: each image (16ch x 256x256) is processed as two halves of
128 rows. A half is laid out as [128 partitions, 4610 cols]:
  partition p = 16*q + c  (q = block 0..7 of 16 image rows, c = channel)
  col u = 257 + block_px   (block_px in [0, 4096), flattened row-major;
                            +-257 halo cols hold wrapped neighbor pixels)

Pipeline per half:
  DMA x -> xt(f32) -> cast xb(bf16)
  conv via shifted-AP DVE TTs: y1 = [1,2,1]_v (x) [-1,0,1]_h, y2 = transpose
    (scales folded into W1 columns on host)
  MLP1: per 512-px chunk, 3 accumulating bf16 matmuls (x, y1, y2 rhs) with
    zero-padded K=32 lhsT, 4-way row-tiled across quadrants -> h psum
  relu+bias evac (ACT) -> h bf16
  MLP2: zero-padded col-tiled matmuls -> dx psum [128,512] in block layout
  evac dx (+b2, *step) -> dxt bf16
Per image: alpha pooling in a [128, 1026] layout (partition = 512-px span),
  life = (min(maxpool(a_pre), maxpool(a_new)) > 0.1), g = life*fire;
  masks redistributed by DMA + stream_shuffle broadcast to block layout.
Tail: out = x*life_bc + dxt*g_bc -> DMA out.
"""

import numpy as np
import ml_dtypes

import concourse.bass as bass
import concourse.mybir as mybir
import concourse.tile as tile
from concourse import bacc

# ---------------- constants ----------------
B, C, H, Wd = 32, 16, 256, 256
PLANE = H * Wd  # 65536
NCORE = 8
NIMG = B // NCORE  # 4 images per core
HALFPX = PLANE // 2  # 32768 px per half (128 rows)
FDH = 4096  # px per block (16 rows)
HALO = 257
XW = HALO + FDH + HALO  # 4610
NJ = FDH // 512  # 8 column chunks per block
ALPHA_CH = 3
ALPHA_THRESH = 0.1
STEP_SIZE = 1.0
HIDDEN = 128

f32 = mybir.dt.float32
bf16 = mybir.dt.bfloat16
i32 = mybir.dt.int32
Alu = mybir.AluOpType
Act = mybir.ActivationFunctionType


def _ap(full: bass.AP, offset_elems: int, dims) -> bass.AP:
    """Build an AP on `full`'s tensor at element offset with explicit dims."""
    return bass.AP(full.tensor, full.offset + offset_elems, [list(d) for d in dims])


def _scols(t: bass.AP, u0: int, step: int, n: int) -> bass.AP:
    """[128, n, 1] AP over strided columns u0 + step*k of a [128, W] tile."""
    full = t[:]
    prow = full.ap[0][0]
    return _ap(full, u0, [[prow, full.ap[0][1]], [step, n], [1, 1]])


def build_kernel(nc: bass.Bass, n_img: int):
    xin = nc.dram_tensor("xin", [n_img, C, PLANE], f32, kind="ExternalInput")
    fire = nc.dram_tensor("fire", [n_img, PLANE], i32, kind="ExternalInput")
    wm1d = nc.dram_tensor("wm1", [128, 768], bf16, kind="ExternalInput")
    wm2d = nc.dram_tensor("wm2", [128, 48], bf16, kind="ExternalInput")
    b1d = nc.dram_tensor("b1t", [128, 1], f32, kind="ExternalInput")
    b2d = nc.dram_tensor("b2t", [128, 1], f32, kind="ExternalInput")
    seld = nc.dram_tensor("selm", [16, 256], bf16, kind="ExternalInput")
    outd = nc.dram_tensor("out", [n_img, C, PLANE], f32, kind="ExternalOutput")

    xin_f = xin.ap()
    fire_f = fire.ap()
    out_f = outd.ap()

    with tile.TileContext(nc) as tc:
        with (
            tc.tile_pool(name="pw", bufs=1) as pw,
            tc.tile_pool(name="pxt", bufs=2) as pxt,
            tc.tile_pool(name="pxb", bufs=1) as pxb,
            tc.tile_pool(name="pcs", bufs=2) as pcs,
            tc.tile_pool(name="py", bufs=1) as py,
            tc.tile_pool(name="phsb", bufs=12) as phsb,
            tc.tile_pool(name="pdxt", bufs=2) as pdxt,
            tc.tile_pool(name="pout", bufs=1) as pout,
            tc.tile_pool(name="pt1", bufs=1) as pt1,

            tc.tile_pool(name="ppl", bufs=1) as ppl,
            tc.tile_pool(name="psh", bufs=4, space="PSUM") as psh,
            tc.tile_pool(name="psb", bufs=1, space="PSUM") as psb,
            tc.tile_pool(name="psd", bufs=2, space="PSUM") as psd,
        ):
            # ---- weights (once) ----
            wm1 = pw.tile([128, 768], bf16)
            nc.sync.dma_start(out=wm1[:], in_=wm1d.ap())
            wm2 = pw.tile([128, 48], bf16)
            nc.sync.dma_start(out=wm2[:], in_=wm2d.ap())
            b1t = pw.tile([128, 1], f32)
            nc.sync.dma_start(out=b1t[:], in_=b1d.ap())
            b2t = pw.tile([128, 1], f32)
            nc.sync.dma_start(out=b2t[:], in_=b2d.ap())
            selm = pw.tile([16, 256], bf16)
            nc.sync.dma_start(out=selm[:], in_=seld.ap())

            for i in range(n_img):
                ibase = i * C * PLANE

                # ============ per-image pool-layout loads ============
                # x3p: alpha plane with +-257 halo; partition p covers
                # px [512p - 257, 512p + 769)
                x3p = ppl.tile([128, 1026], f32, name="x3p", tag="x3p", bufs=1)
                abase = ibase + ALPHA_CH * PLANE
                nc.sync.dma_start(
                    out=x3p[1:127, :],
                    in_=_ap(xin_f, abase + 512 - 257, [[512, 126], [1, 1026]]),
                )
                nc.sync.dma_start(
                    out=x3p[0:1, 257:1026],
                    in_=_ap(xin_f, abase, [[769, 1], [1, 769]]),
                )
                nc.sync.dma_start(
                    out=x3p[0:1, 0:257],
                    in_=_ap(xin_f, abase + PLANE - 257, [[257, 1], [1, 257]]),
                )
                nc.sync.dma_start(
                    out=x3p[127:128, 0:769],
                    in_=_ap(xin_f, abase + 512 * 127 - 257, [[769, 1], [1, 769]]),
                )
                nc.sync.dma_start(
                    out=x3p[127:128, 769:1026],
                    in_=_ap(xin_f, abase, [[257, 1], [1, 257]]),
                )

                mt = ppl.tile([128, 512], i32, name="mt", tag="mt", bufs=1)
                nc.sync.dma_start(
                    out=mt[:], in_=_ap(fire_f, i * PLANE, [[512, 128], [1, 512]])
                )
                mbf = ppl.tile([128, 512], bf16, name="mbf", tag="mbf", bufs=1)
                nc.vector.tensor_copy(out=mbf[:], in_=mt[:])

                # pre-life maxpool on x3p
                pre = self_pool_max(nc, ppl, x3p, "pre")

                dx3p = ppl.tile([128, 512], bf16, name="dx3p", tag="dx3p", bufs=1)
                dxts = []
                xts = []
                for h in range(2):
                    hbase = ibase + HALFPX * h

                    # ============ load x half ============
                    xt = pxt.tile([128, XW], f32, name="xt", tag="xt")
                    xts.append(xt)
                    nc.sync.dma_start(
                        out=xt[:, HALO : HALO + FDH],
                        in_=_ap(xin_f, hbase, [[FDH, 8], [PLANE, 16], [1, FDH]]),
                    )
                    # left halo
                    if h == 0:
                        nc.sync.dma_start(
                            out=xt[16:128, 0:HALO],
                            in_=_ap(
                                xin_f, ibase + FDH - HALO,
                                [[FDH, 7], [PLANE, 16], [1, HALO]],
                            ),
                        )
                        nc.sync.dma_start(
                            out=xt[0:16, 0:HALO],
                            in_=_ap(xin_f, ibase + PLANE - HALO, [[PLANE, 16], [1, HALO]]),
                        )
                    else:
                        nc.sync.dma_start(
                            out=xt[:, 0:HALO],
                            in_=_ap(
                                xin_f, hbase - HALO,
                                [[FDH, 8], [PLANE, 16], [1, HALO]],
                            ),
                        )
                    # right halo
                    if h == 0:
                        nc.sync.dma_start(
                            out=xt[:, HALO + FDH : XW],
                            in_=_ap(
                                xin_f, hbase + FDH, [[FDH, 8], [PLANE, 16], [1, HALO]]
                            ),
                        )
                    else:
                        nc.sync.dma_start(
                            out=xt[0:112, HALO + FDH : XW],
                            in_=_ap(
                                xin_f, hbase + FDH, [[FDH, 7], [PLANE, 16], [1, HALO]]
                            ),
                        )
                        nc.sync.dma_start(
                            out=xt[112:128, HALO + FDH : XW],
                            in_=_ap(xin_f, ibase, [[PLANE, 16], [1, HALO]]),
                        )

                    # ============ conv (bf16) ============
                    xb = pxb.tile([128, XW], bf16, name="xb", tag="xb")
                    nc.vector.tensor_copy(out=xb[:], in_=xt[:])

                    pt = pcs.tile([128, XW], bf16, name="csA", tag="cs")
                    # p = xb(u+1) - xb(u-1) on [1, 4609)
                    nc.vector.tensor_sub(
                        out=pt[:, 1 : XW - 1], in0=xb[:, 2:XW], in1=xb[:, 0 : XW - 2]
                    )
                    # fix j=0 cols (u = 1 + 256k): p[u] = xb[u+1] - xb[u+255]
                    nfix = (XW - 2 - 1) // 256 + 1  # 18
                    nc.vector.tensor_sub(
                        out=_scols(pt, 1, 256, nfix),
                        in0=_scols(xb, 2, 256, nfix),
                        in1=_scols(xb, 256, 256, nfix),
                    )
                    # fix j=255 cols (u = 256k): p[u] = xb[u-255] - xb[u-1]
                    nc.vector.tensor_sub(
                        out=_scols(pt, 256, 256, nfix),
                        in0=_scols(xb, 1, 256, nfix),
                        in1=_scols(xb, 255, 256, nfix),
                    )
                    # p2 = p + p(+256) on [1, 4353)
                    p2 = pcs.tile([128, XW], bf16, name="csB", tag="cs")
                    nc.vector.tensor_add(
                        out=p2[:, 1 : HALO + FDH],
                        in0=pt[:, 1 : HALO + FDH],
                        in1=pt[:, 257 : HALO + FDH + 256],
                    )
                    # y1 = p2(u) + p2(u-256), valid block px [0, 4096)
                    y1 = py.tile([128, FDH], bf16, name="y1", tag="y1")
                    nc.vector.tensor_add(
                        out=y1[:],
                        in0=p2[:, HALO : HALO + FDH],
                        in1=p2[:, 1 : 1 + FDH],
                    )
                    # s1 = xb(u) + xb(u+1) on [0, 4609)
                    s1 = pcs.tile([128, XW], bf16, name="csC", tag="cs")
                    nc.vector.tensor_add(
                        out=s1[:, 0 : XW - 1], in0=xb[:, 0 : XW - 1], in1=xb[:, 1:XW]
                    )
                    # s2 = s1(u) + s1(u-1) on [1, 4609)
                    s2 = pcs.tile([128, XW], bf16, name="csD", tag="cs")
                    nc.vector.tensor_add(
                        out=s2[:, 1 : XW - 1], in0=s1[:, 1 : XW - 1], in1=s1[:, 0 : XW - 2]
                    )
                    # fix s2 at j=0 (u = 1+256k): s2 = xb[u+255] + 2 xb[u] + xb[u+1]
                    tfx = pcs.tile([128, 32], bf16, name="tfx", tag="tfx", bufs=2)
                    nc.vector.tensor_add(
                        out=_scols(tfx, 0, 1, nfix),
                        in0=_scols(xb, 256, 256, nfix),
                        in1=_scols(xb, 2, 256, nfix),
                    )
                    nc.vector.scalar_tensor_tensor(
                        out=_scols(s2, 1, 256, nfix),
                        in0=_scols(xb, 1, 256, nfix),
                        scalar=2.0,
                        in1=_scols(tfx, 0, 1, nfix),
                        op0=Alu.mult,
                        op1=Alu.add,
                    )
                    # fix s2 at j=255 (u = 256k): s2 = xb[u-255] + 2 xb[u] + xb[u-1]
                    tfx2 = pcs.tile([128, 32], bf16, name="tfx2", tag="tfx", bufs=2)
                    nc.vector.tensor_add(
                        out=_scols(tfx2, 0, 1, nfix),
                        in0=_scols(xb, 1, 256, nfix),
                        in1=_scols(xb, 255, 256, nfix),
                    )
                    nc.vector.scalar_tensor_tensor(
                        out=_scols(s2, 256, 256, nfix),
                        in0=_scols(xb, 256, 256, nfix),
                        scalar=2.0,
                        in1=_scols(tfx2, 0, 1, nfix),
                        op0=Alu.mult,
                        op1=Alu.add,
                    )
                    # y2 = s2(u+256) - s2(u-256), valid block px [0, 4096)
                    y2 = py.tile([128, FDH], bf16, name="y2", tag="y2")
                    nc.vector.tensor_sub(
                        out=y2[:],
                        in0=s2[:, HALO + 256 : HALO + 256 + FDH],
                        in1=s2[:, 1 : 1 + FDH],
                    )

                    # ============ MLP over 512-px chunks ============
                    dxt = pdxt.tile([128, FDH], bf16, name="dxt", tag="dxt")
                    dxts.append(dxt)
                    for j in range(NJ):
                        c0 = 512 * j
                        hsbs = []
                        for u in (0, 1):  # A chunks (even q) then B chunks
                            hpss = []
                            for s in range(4):
                                hps = psh.tile(
                                    [128, 512], f32, name=f"hps{u}{s}", tag="hps"
                                )
                                hpss.append(hps)
                                for k, src in ((0, None), (1, y1), (2, y2)):
                                    if k == 0:
                                        rhs = xb[
                                            32 * s : 32 * s + 32,
                                            HALO + c0 : HALO + c0 + 512,
                                        ]
                                    else:
                                        rhs = src[32 * s : 32 * s + 32, c0 : c0 + 512]
                                    nc.tensor.matmul(
                                        out=hps[:],
                                        lhsT=wm1[
                                            32 * s : 32 * s + 32,
                                            128 * (2 * k + u) : 128 * (2 * k + u) + 128,
                                        ],
                                        rhs=rhs,
                                        start=(k == 0),
                                        stop=(k == 2),
                                        tile_position=(32 * s, 0),
                                    )
                            for s in range(4):
                                hsb = phsb.tile([128, 512], bf16, name=f"hsb{u}{s}", tag="hsb")
                                nc.scalar.activation(
                                    out=hsb[:],
                                    in_=hpss[s][:],
                                    func=Act.Relu,
                                    bias=b1t[:, 0:1],
                                    scale=1.0,
                                )
                                hsbs.append((2 * s + u, hsb))
                        hmap = dict(hsbs)
                        dxps = psd.tile([128, 512], f32, name="dxps", tag="dxps")
                        for s in range(4):
                            nc.tensor.matmul(
                                out=dxps[32 * s : 32 * s + 32, :],
                                lhsT=wm2[:, 16:48],
                                rhs=hmap[2 * s + 1][:],
                                start=True,
                                stop=False,
                                skip_group_check=True,
                                tile_position=(0, 32 * s),
                            )
                            nc.tensor.matmul(
                                out=dxps[32 * s : 32 * s + 16, :],
                                lhsT=wm2[:, 0:16],
                                rhs=hmap[2 * s][:],
                                start=False,
                                stop=True,
                                skip_group_check=True,
                                tile_position=(0, 32 * s),
                            )
                        nc.scalar.activation(
                            out=dxt[:, c0 : c0 + 512],
                            in_=dxps[:],
                            func=Act.Identity,
                            bias=b2t[:, 0:1],
                            scale=STEP_SIZE,
                        )

                    # extract dx alpha rows into pool layout
                    # dst partitions 64h+8q+sub <- dxt[3 + 16q, 512*sub + px]
                    for q in range(8):
                        nc.sync.dma_start(
                            out=dx3p[64 * h + 8 * q : 64 * h + 8 * q + 8, :],
                            in_=_ap(
                                dxt[:], (3 + 16 * q) * FDH,
                                [[FDH, 1], [512, 8], [1, 512]],
                            ),
                        )

                # ============ per-image pooling / masks ============
                tmask = ppl.tile([128, 512], bf16, name="tmask", tag="tmask", bufs=1)
                nc.vector.tensor_mul(out=tmask[:], in0=dx3p[:], in1=mbf[:])
                anp = ppl.tile([128, 1026], f32, name="anp", tag="anp")
                nc.vector.tensor_add(
                    out=anp[:, 257:769], in0=x3p[:, 257:769], in1=tmask[:]
                )
                # halo gather for anp
                nc.sync.dma_start(out=anp[1:128, 0:257], in_=anp[0:127, 512:769])
                nc.sync.dma_start(out=anp[0:1, 0:257], in_=anp[127:128, 512:769])
                nc.sync.dma_start(out=anp[0:127, 769:1026], in_=anp[1:128, 257:514])
                nc.sync.dma_start(out=anp[127:128, 769:1026], in_=anp[0:1, 257:514])
                post = self_pool_max(nc, ppl, anp, "post")

                nc.vector.tensor_tensor(
                    out=pre[:], in0=pre[:], in1=post[:], op=Alu.min
                )
                life = ppl.tile([128, 512], bf16, name="life", tag="life", bufs=1)
                nc.vector.tensor_scalar(
                    out=life[:], in0=pre[:], scalar1=ALPHA_THRESH, scalar2=None,
                    op0=Alu.is_gt,
                )
                gm = ppl.tile([128, 512], bf16, name="gm", tag="gm", bufs=1)
                nc.vector.tensor_mul(out=gm[:], in0=life[:], in1=mbf[:])

                # compact masks to [16, 4096] row-per-block layout
                liferow = ppl.tile([128, FDH], bf16, name="liferow", tag="liferow")
                gmrow = ppl.tile([128, FDH], bf16, name="gmrow", tag="gmrow")
                nc.sync.dma_start(out=liferow[0:16, :], in_=life[:])
                nc.sync.dma_start(out=gmrow[0:16, :], in_=gm[:])

                # ============ per-half mask expand (PE) + tail ============
                for h in range(2):
                    xt = xts[h]
                    dxt = dxts[h]
                    out_t = pout.tile([128, FDH], f32, name="ot", tag="ot")
                    for jc in range(NJ):
                        c0 = 512 * jc
                        bclp = psb.tile([128, 512], f32, name="bclp", tag="bclp")
                        nc.tensor.matmul(
                            out=bclp[:],
                            lhsT=selm[0:16, 128 * h : 128 * h + 128],
                            rhs=liferow[0:16, c0 : c0 + 512],
                            start=True,
                            stop=True,
                            tile_position=(0, 0),
                        )
                        bcgp = psb.tile([128, 512], f32, name="bcgp", tag="bcgp")
                        nc.tensor.matmul(
                            out=bcgp[:],
                            lhsT=selm[0:16, 128 * h : 128 * h + 128],
                            rhs=gmrow[0:16, c0 : c0 + 512],
                            start=True,
                            stop=True,
                            tile_position=(0, 0),
                        )
                        t1 = pt1.tile([128, 512], bf16, name="t1", tag="t1", bufs=4)
                        nc.vector.tensor_mul(
                            out=t1[:], in0=dxt[:, c0 : c0 + 512], in1=bcgp[:]
                        )
                        nc.vector.tensor_mul(
                            out=out_t[:, c0 : c0 + 512],
                            in0=xt[:, HALO + c0 : HALO + c0 + 512],
                            in1=bclp[:],
                        )
                        nc.vector.tensor_add(
                            out=out_t[:, c0 : c0 + 512],
                            in0=out_t[:, c0 : c0 + 512],
                            in1=t1[:],
                        )
                    nc.sync.dma_start(
                        out=_ap(
                            out_f, ibase + HALFPX * h,
                            [[FDH, 8], [PLANE, 16], [1, FDH]],
                        ),
                        in_=out_t[:],
                    )
    return nc


def self_pool_max(nc, ppl, src, name):
    """3x3 wrap max-pool of a [128, 1026] pool-layout alpha tile.

    Returns [128, 512] tile of pooled values for the valid 512 px.
    Pool layout: partition p covers px [512p - 257, 512p + 769); flat index,
    image col j = (col - 1) mod 256.
    """
    mh = ppl.tile([128, 1026], f32, name=f"mh_{name}", tag="mh", bufs=1)
    # horizontal 3-max on [1, 1025)
    nc.vector.tensor_max(out=mh[:, 1:1025], in0=src[:, 0:1024], in1=src[:, 1:1025])
    nc.vector.tensor_max(out=mh[:, 1:1025], in0=mh[:, 1:1025], in1=src[:, 2:1026])
    # fix j=0 cols {1, 257, 513, 769}: max(src[c], src[c+1], src[c+255])
    nc.vector.tensor_max(
        out=_scols(mh, 1, 256, 4), in0=_scols(src, 1, 256, 4), in1=_scols(src, 2, 256, 4)
    )
    nc.vector.tensor_max(
        out=_scols(mh, 1, 256, 4), in0=_scols(mh, 1, 256, 4), in1=_scols(src, 256, 256, 4)
    )
    # fix j=255 cols {256, 512, 768}: max(src[c-1], src[c], src[c-255])
    nc.vector.tensor_max(
        out=_scols(mh, 256, 256, 3),
        in0=_scols(src, 255, 256, 3),
        in1=_scols(src, 256, 256, 3),
    )
    nc.vector.tensor_max(
        out=_scols(mh, 256, 256, 3),
        in0=_scols(mh, 256, 256, 3),
        in1=_scols(src, 1, 256, 3),
    )
    # vertical 3-max -> valid [257, 769)
    out = ppl.tile([128, 512], f32, name=f"pool_{name}", tag=f"po_{name}", bufs=1)
    nc.vector.tensor_max(out=out[:], in0=mh[:, 1:513], in1=mh[:, 257:769])
    nc.vector.tensor_max(out=out[:], in0=out[:], in1=mh[:, 513:1025])
    return out


def _host_weights(w1, b1, w2, b2):
    w1 = np.asarray(w1, np.float32)
    w2 = np.asarray(w2, np.float32)
    b1 = np.asarray(b1, np.float32)
    b2 = np.asarray(b2, np.float32)
    W1k = [w1[:, 0::3], w1[:, 1::3] * 0.125, w1[:, 2::3] * 0.125]
    wm1 = np.zeros((128, 768), np.float32)
    for s in range(4):
        for k in range(3):
            blkA = 128 * (2 * k)
            blkB = 128 * (2 * k + 1)
            wm1[32 * s : 32 * s + 16, blkA : blkA + 128] = W1k[k].T
            wm1[32 * s + 16 : 32 * s + 32, blkB : blkB + 128] = W1k[k].T
    wm2 = np.zeros((128, 48), np.float32)
    wm2[:, 0:16] = w2.T
    wm2[:, 32:48] = w2.T
    b1t = b1.reshape(128, 1)
    b2t = np.tile(b2 * STEP_SIZE, 8).reshape(128, 1)
    selm = np.zeros((16, 256), np.float32)
    for hh in range(2):
        for q in range(8):
            for c in range(16):
                selm[8 * hh + q, 128 * hh + 16 * q + c] = 1.0
    return (
        wm1.astype(ml_dtypes.bfloat16),
        wm2.astype(ml_dtypes.bfloat16),
        b1t.astype(np.float32),
        b2t.astype(np.float32),
        selm.astype(ml_dtypes.bfloat16),
    )


_NC_CACHE = {}


def _get_nc(n_img):
    if n_img not in _NC_CACHE:
        nc = bacc.Bacc("TRN2", target_bir_lowering=False, debug=False)
        build_kernel(nc, n_img)
        nc.compile()
        _NC_CACHE[n_img] = nc
    return _NC_CACHE[n_img]


def kernel(x, w1, b1, w2, b2, fire_mask):
    from concourse.bass_utils import run_bass_kernel_spmd

    x = np.asarray(x, np.float32).reshape(B, C, PLANE)
    fire = np.asarray(fire_mask, np.int32).reshape(B, PLANE)
    wm1, wm2, b1t, b2t, selm = _host_weights(w1, b1, w2, b2)

    nc = _get_nc(NIMG)
    in_maps = []
    for core in range(NCORE):
        sl = slice(core * NIMG, (core + 1) * NIMG)
        in_maps.append(
            {
                "xin": np.ascontiguousarray(x[sl]),
                "fire": np.ascontiguousarray(fire[sl]),
                "wm1": wm1,
                "wm2": wm2,
                "b1t": b1t,
                "b2t": b2t,
                "selm": selm,
            }
        )
    res = run_bass_kernel_spmd(nc, in_maps, core_ids=list(range(NCORE)))
    outs = [res.results[c]["out"].reshape(NIMG, C, H, Wd) for c in range(NCORE)]
    return np.concatenate(outs, axis=0)


# revision 12
# speedup vs baseline: 1.4072x; 1.4072x over previous
"""Trainium2 Bass kernel for nn_CAModel (neural cellular automata step).

Data-parallel over 8 NeuronCores: 4 images per core.

Per-core layout: each image (16ch x 256x256) is processed as two halves of
128 rows. A half is laid out as [128 partitions, 4610 cols]:
  partition p = 16*q + c  (q = block 0..7 of 16 image rows, c = channel)
  col u = 257 + block_px   (block_px in [0, 4096), flattened row-major;
                            +-257 halo cols hold wrapped neighbor pixels)

Pipeline per half:
  DMA x -> xt(f32) -> cast xb(bf16)
  conv via shifted-AP DVE TTs: y1 = [1,2,1]_v (x) [-1,0,1]_h, y2 = transpose
    (scales folded into W1 columns on host)
  MLP1: per 512-px chunk, 3 accumulating bf16 matmuls (x, y1, y2 rhs) with
    zero-padded K=32 lhsT, 4-way row-tiled across quadrants -> h psum
  relu+bias evac (ACT) -> h bf16
  MLP2: zero-padded col-tiled matmuls -> dx psum [128,512] in block layout
  evac dx (+b2, *step) -> dxt bf16
Per image: alpha pooling in a [128, 1026] layout (partition = 512-px span),
  life = (min(maxpool(a_pre), maxpool(a_new)) > 0.1), g = life*fire;
  masks redistributed by DMA + stream_shuffle broadcast to block layout.
Tail: out = x*life_bc + dxt*g_bc -> DMA out.
"""

import numpy as np
import ml_dtypes

import concourse.bass as bass
import concourse.mybir as mybir
import concourse.tile as tile
from concourse import bacc

# ---------------- constants ----------------
B, C, H, Wd = 32, 16, 256, 256
PLANE = H * Wd  # 65536
NCORE = 8
NIMG = B // NCORE  # 4 images per core
HALFPX = PLANE // 2  # 32768 px per half (128 rows)
FDH = 4096  # px per block (16 rows)
HALO = 257
XW = HALO + FDH + HALO  # 4610
NJ = FDH // 512  # 8 column chunks per block
ALPHA_CH = 3
ALPHA_THRESH = 0.1
STEP_SIZE = 1.0
HIDDEN = 128

f32 = mybir.dt.float32
bf16 = mybir.dt.bfloat16
i32 = mybir.dt.int32
Alu = mybir.AluOpType
Act = mybir.ActivationFunctionType


def _ap(full: bass.AP, offset_elems: int, dims) -> bass.AP:
    """Build an AP on `full`'s tensor at element offset with explicit dims."""
    return bass.AP(full.tensor, full.offset + offset_elems, [list(d) for d in dims])


def _scols(t: bass.AP, u0: int, step: int, n: int) -> bass.AP:
    """[128, n, 1] AP over strided columns u0 + step*k of a [128, W] tile."""
    full = t[:]
    prow = full.ap[0][0]
    return _ap(full, u0, [[prow, full.ap[0][1]], [step, n], [1, 1]])


def build_kernel(nc: bass.Bass, n_img: int):
    xin = nc.dram_tensor("xin", [n_img, C, PLANE], f32, kind="ExternalInput")
    fire = nc.dram_tensor("fire", [n_img, PLANE], i32, kind="ExternalInput")
    wm1d = nc.dram_tensor("wm1", [128, 768], bf16, kind="ExternalInput")
    wm2d = nc.dram_tensor("wm2", [128, 48], bf16, kind="ExternalInput")
    b1d = nc.dram_tensor("b1t", [128, 1], f32, kind="ExternalInput")
    b2d = nc.dram_tensor("b2t", [128, 1], f32, kind="ExternalInput")
    seld = nc.dram_tensor("selm", [16, 256], bf16, kind="ExternalInput")
    outd = nc.dram_tensor("out", [n_img, C, PLANE], f32, kind="ExternalOutput")

    xin_f = xin.ap()
    fire_f = fire.ap()
    out_f = outd.ap()

    with tile.TileContext(nc) as tc:
        with (
            tc.tile_pool(name="pw", bufs=1) as pw,
            tc.tile_pool(name="pxt", bufs=2) as pxt,
            tc.tile_pool(name="pxb", bufs=1) as pxb,
            tc.tile_pool(name="pcs", bufs=2) as pcs,
            tc.tile_pool(name="py", bufs=1) as py,
            tc.tile_pool(name="phsb", bufs=12) as phsb,
            tc.tile_pool(name="pdxt", bufs=2) as pdxt,
            tc.tile_pool(name="pout", bufs=1) as pout,
            tc.tile_pool(name="pt1", bufs=1) as pt1,

            tc.tile_pool(name="ppl", bufs=1) as ppl,
            tc.tile_pool(name="psh", bufs=4, space="PSUM") as psh,
            tc.tile_pool(name="psb", bufs=1, space="PSUM") as psb,
            tc.tile_pool(name="psd", bufs=2, space="PSUM") as psd,
        ):
            # ---- weights (once) ----
            wm1 = pw.tile([128, 768], bf16)
            nc.sync.dma_start(out=wm1[:], in_=wm1d.ap())
            wm2 = pw.tile([128, 48], bf16)
            nc.sync.dma_start(out=wm2[:], in_=wm2d.ap())
            b1t = pw.tile([128, 1], f32)
            nc.sync.dma_start(out=b1t[:], in_=b1d.ap())
            b2t = pw.tile([128, 1], f32)
            nc.sync.dma_start(out=b2t[:], in_=b2d.ap())
            selm = pw.tile([16, 256], bf16)
            nc.sync.dma_start(out=selm[:], in_=seld.ap())

            for i in range(n_img):
                ibase = i * C * PLANE

                # ============ per-image pool-layout loads ============
                # x3p: alpha plane with +-257 halo; partition p covers
                # px [512p - 257, 512p + 769)
                x3p = ppl.tile([128, 1026], f32, name="x3p", tag="x3p", bufs=1)
                abase = ibase + ALPHA_CH * PLANE
                nc.sync.dma_start(
                    out=x3p[1:127, :],
                    in_=_ap(xin_f, abase + 512 - 257, [[512, 126], [1, 1026]]),
                )
                nc.sync.dma_start(
                    out=x3p[0:1, 257:1026],
                    in_=_ap(xin_f, abase, [[769, 1], [1, 769]]),
                )
                nc.sync.dma_start(
                    out=x3p[0:1, 0:257],
                    in_=_ap(xin_f, abase + PLANE - 257, [[257, 1], [1, 257]]),
                )
                nc.sync.dma_start(
                    out=x3p[127:128, 0:769],
                    in_=_ap(xin_f, abase + 512 * 127 - 257, [[769, 1], [1, 769]]),
                )
                nc.sync.dma_start(
                    out=x3p[127:128, 769:1026],
                    in_=_ap(xin_f, abase, [[257, 1], [1, 257]]),
                )

                mt = ppl.tile([128, 512], i32, name="mt", tag="mt", bufs=1)
                nc.sync.dma_start(
                    out=mt[:], in_=_ap(fire_f, i * PLANE, [[512, 128], [1, 512]])
                )
                mbf = ppl.tile([128, 512], bf16, name="mbf", tag="mbf", bufs=1)
                nc.vector.tensor_copy(out=mbf[:], in_=mt[:])

                # pre-life maxpool on x3p
                pre = self_pool_max(nc, ppl, x3p, "pre")

                dx3p = ppl.tile([128, 512], bf16, name="dx3p", tag="dx3p", bufs=1)
                dxts = []
                xts = []
                for h in range(2):
                    hbase = ibase + HALFPX * h

                    # ============ load x half ============
                    xt = pxt.tile([128, XW], f32, name="xt", tag="xt")
                    xts.append(xt)
                    nc.sync.dma_start(
                        out=xt[:, HALO : HALO + FDH],
                        in_=_ap(xin_f, hbase, [[FDH, 8], [PLANE, 16], [1, FDH]]),
                    )
                    # left halo
                    if h == 0:
                        nc.sync.dma_start(
                            out=xt[16:128, 0:HALO],
                            in_=_ap(
                                xin_f, ibase + FDH - HALO,
                                [[FDH, 7], [PLANE, 16], [1, HALO]],
                            ),
                        )
                        nc.sync.dma_start(
                            out=xt[0:16, 0:HALO],
                            in_=_ap(xin_f, ibase + PLANE - HALO, [[PLANE, 16], [1, HALO]]),
                        )
                    else:
                        nc.sync.dma_start(
                            out=xt[:, 0:HALO],
                            in_=_ap(
                                xin_f, hbase - HALO,
                                [[FDH, 8], [PLANE, 16], [1, HALO]],
                            ),
                        )
                    # right halo
                    if h == 0:
                        nc.sync.dma_start(
                            out=xt[:, HALO + FDH : XW],
                            in_=_ap(
                                xin_f, hbase + FDH, [[FDH, 8], [PLANE, 16], [1, HALO]]
                            ),
                        )
                    else:
                        nc.sync.dma_start(
                            out=xt[0:112, HALO + FDH : XW],
                            in_=_ap(
                                xin_f, hbase + FDH, [[FDH, 7], [PLANE, 16], [1, HALO]]
                            ),
                        )
                        nc.sync.dma_start(
                            out=xt[112:128, HALO + FDH : XW],
                            in_=_ap(xin_f, ibase, [[PLANE, 16], [1, HALO]]),
                        )

                    # ============ conv (bf16) ============
                    xb = pxb.tile([128, XW], bf16, name="xb", tag="xb")
                    nc.vector.tensor_copy(out=xb[:], in_=xt[:])

                    pt = pcs.tile([128, XW], bf16, name="csA", tag="cs")
                    # p = xb(u+1) - xb(u-1) on [1, 4609)
                    nc.vector.tensor_sub(
                        out=pt[:, 1 : XW - 1], in0=xb[:, 2:XW], in1=xb[:, 0 : XW - 2]
                    )
                    # fix j=0 cols (u = 1 + 256k): p[u] = xb[u+1] - xb[u+255]
                    nfix = (XW - 2 - 1) // 256 + 1  # 18
                    nc.vector.tensor_sub(
                        out=_scols(pt, 1, 256, nfix),
                        in0=_scols(xb, 2, 256, nfix),
                        in1=_scols(xb, 256, 256, nfix),
                    )
                    # fix j=255 cols (u = 256k): p[u] = xb[u-255] - xb[u-1]
                    nc.vector.tensor_sub(
                        out=_scols(pt, 256, 256, nfix),
                        in0=_scols(xb, 1, 256, nfix),
                        in1=_scols(xb, 255, 256, nfix),
                    )
                    # p2 = p + p(+256) on [1, 4353)
                    p2 = pcs.tile([128, XW], bf16, name="csB", tag="cs")
                    nc.vector.tensor_add(
                        out=p2[:, 1 : HALO + FDH],
                        in0=pt[:, 1 : HALO + FDH],
                        in1=pt[:, 257 : HALO + FDH + 256],
                    )
                    # y1 = p2(u) + p2(u-256), valid block px [0, 4096)
                    y1 = py.tile([128, FDH], bf16, name="y1", tag="y1")
                    nc.vector.tensor_add(
                        out=y1[:],
                        in0=p2[:, HALO : HALO + FDH],
                        in1=p2[:, 1 : 1 + FDH],
                    )
                    # s1 = xb(u) + xb(u+1) on [0, 4609)
                    s1 = pcs.tile([128, XW], bf16, name="csC", tag="cs")
                    nc.vector.tensor_add(
                        out=s1[:, 0 : XW - 1], in0=xb[:, 0 : XW - 1], in1=xb[:, 1:XW]
                    )
                    # s2 = s1(u) + s1(u-1) on [1, 4609)
                    s2 = pcs.tile([128, XW], bf16, name="csD", tag="cs")
                    nc.vector.tensor_add(
                        out=s2[:, 1 : XW - 1], in0=s1[:, 1 : XW - 1], in1=s1[:, 0 : XW - 2]
                    )
                    # fix s2 at j=0 (u = 1+256k): s2 = xb[u+255] + 2 xb[u] + xb[u+1]
                    tfx = pcs.tile([128, 32], bf16, name="tfx", tag="tfx", bufs=2)
                    nc.vector.tensor_add(
                        out=_scols(tfx, 0, 1, nfix),
                        in0=_scols(xb, 256, 256, nfix),
                        in1=_scols(xb, 2, 256, nfix),
                    )
                    nc.vector.scalar_tensor_tensor(
                        out=_scols(s2, 1, 256, nfix),
                        in0=_scols(xb, 1, 256, nfix),
                        scalar=2.0,
                        in1=_scols(tfx, 0, 1, nfix),
                        op0=Alu.mult,
                        op1=Alu.add,
                    )
                    # fix s2 at j=255 (u = 256k): s2 = xb[u-255] + 2 xb[u] + xb[u-1]
                    tfx2 = pcs.tile([128, 32], bf16, name="tfx2", tag="tfx", bufs=2)
                    nc.vector.tensor_add(
                        out=_scols(tfx2, 0, 1, nfix),
                        in0=_scols(xb, 1, 256, nfix),
                        in1=_scols(xb, 255, 256, nfix),
                    )
                    nc.vector.scalar_tensor_tensor(
                        out=_scols(s2, 256, 256, nfix),
                        in0=_scols(xb, 256, 256, nfix),
                        scalar=2.0,
                        in1=_scols(tfx2, 0, 1, nfix),
                        op0=Alu.mult,
                        op1=Alu.add,
                    )
                    # y2 = s2(u+256) - s2(u-256), valid block px [0, 4096)
                    y2 = py.tile([128, FDH], bf16, name="y2", tag="y2")
                    nc.vector.tensor_sub(
                        out=y2[:],
                        in0=s2[:, HALO + 256 : HALO + 256 + FDH],
                        in1=s2[:, 1 : 1 + FDH],
                    )

                    # ============ MLP over 512-px chunks ============
                    dxt = pdxt.tile([128, FDH], bf16, name="dxt", tag="dxt")
                    dxts.append(dxt)
                    for j in range(NJ):
                        c0 = 512 * j
                        hsbs = []
                        for u in (0, 1):  # A chunks (even q) then B chunks
                            hpss = []
                            for s in range(4):
                                hps = psh.tile(
                                    [128, 512], f32, name=f"hps{u}{s}", tag="hps"
                                )
                                hpss.append(hps)
                                for k, src in ((0, None), (1, y1), (2, y2)):
                                    if k == 0:
                                        rhs = xb[
                                            32 * s : 32 * s + 32,
                                            HALO + c0 : HALO + c0 + 512,
                                        ]
                                    else:
                                        rhs = src[32 * s : 32 * s + 32, c0 : c0 + 512]
                                    nc.tensor.matmul(
                                        out=hps[:],
                                        lhsT=wm1[
                                            32 * s : 32 * s + 32,
                                            128 * (2 * k + u) : 128 * (2 * k + u) + 128,
                                        ],
                                        rhs=rhs,
                                        start=(k == 0),
                                        stop=(k == 2),
                                        tile_position=(32 * s, 0),
                                    )
                            for s in range(4):
                                hsb = phsb.tile([128, 512], bf16, name=f"hsb{u}{s}", tag="hsb")
                                if u == 1 and s < 2:
                                    nc.vector.tensor_scalar(
                                        out=hsb[:],
                                        in0=hpss[s][:],
                                        scalar1=b1t[:, 0:1],
                                        scalar2=0.0,
                                        op0=Alu.add,
                                        op1=Alu.max,
                                    )
                                else:
                                    nc.scalar.activation(
                                        out=hsb[:],
                                        in_=hpss[s][:],
                                        func=Act.Relu,
                                        bias=b1t[:, 0:1],
                                        scale=1.0,
                                    )
                                hsbs.append((2 * s + u, hsb))
                        hmap = dict(hsbs)
                        dxps = psd.tile([128, 512], f32, name="dxps", tag="dxps")
                        for s in range(4):
                            nc.tensor.matmul(
                                out=dxps[32 * s : 32 * s + 32, :],
                                lhsT=wm2[:, 16:48],
                                rhs=hmap[2 * s + 1][:],
                                start=True,
                                stop=False,
                                skip_group_check=True,
                                tile_position=(0, 32 * s),
                            )
                            nc.tensor.matmul(
                                out=dxps[32 * s : 32 * s + 16, :],
                                lhsT=wm2[:, 0:16],
                                rhs=hmap[2 * s][:],
                                start=False,
                                stop=True,
                                skip_group_check=True,
                                tile_position=(0, 32 * s),
                            )
                        nc.scalar.activation(
                            out=dxt[:, c0 : c0 + 512],
                            in_=dxps[:],
                            func=Act.Identity,
                            bias=b2t[:, 0:1],
                            scale=STEP_SIZE,
                        )

                    # extract dx alpha rows into pool layout
                    # dst partitions 64h+8q+sub <- dxt[3 + 16q, 512*sub + px]
                    for q in range(8):
                        nc.sync.dma_start(
                            out=dx3p[64 * h + 8 * q : 64 * h + 8 * q + 8, :],
                            in_=_ap(
                                dxt[:], (3 + 16 * q) * FDH,
                                [[FDH, 1], [512, 8], [1, 512]],
                            ),
                        )

                # ============ per-image pooling / masks ============
                tmask = ppl.tile([128, 512], bf16, name="tmask", tag="tmask", bufs=1)
                nc.vector.tensor_mul(out=tmask[:], in0=dx3p[:], in1=mbf[:])
                anp = ppl.tile([128, 1026], f32, name="anp", tag="anp")
                nc.vector.tensor_add(
                    out=anp[:, 257:769], in0=x3p[:, 257:769], in1=tmask[:]
                )
                # halo gather for anp
                nc.sync.dma_start(out=anp[1:128, 0:257], in_=anp[0:127, 512:769])
                nc.sync.dma_start(out=anp[0:1, 0:257], in_=anp[127:128, 512:769])
                nc.sync.dma_start(out=anp[0:127, 769:1026], in_=anp[1:128, 257:514])
                nc.sync.dma_start(out=anp[127:128, 769:1026], in_=anp[0:1, 257:514])
                post = self_pool_max(nc, ppl, anp, "post")

                nc.vector.tensor_tensor(
                    out=pre[:], in0=pre[:], in1=post[:], op=Alu.min
                )
                life = ppl.tile([128, 512], bf16, name="life", tag="life", bufs=1)
                nc.vector.tensor_scalar(
                    out=life[:], in0=pre[:], scalar1=ALPHA_THRESH, scalar2=None,
                    op0=Alu.is_gt,
                )
                gm = ppl.tile([128, 512], bf16, name="gm", tag="gm", bufs=1)
                nc.vector.tensor_mul(out=gm[:], in0=life[:], in1=mbf[:])

                # compact masks to [16, 4096] row-per-block layout
                liferow = ppl.tile([128, FDH], bf16, name="liferow", tag="liferow")
                gmrow = ppl.tile([128, FDH], bf16, name="gmrow", tag="gmrow")
                nc.sync.dma_start(out=liferow[0:16, :], in_=life[:])
                nc.sync.dma_start(out=gmrow[0:16, :], in_=gm[:])

                # ============ per-half mask expand (PE) + tail ============
                for h in range(2):
                    xt = xts[h]
                    dxt = dxts[h]
                    out_t = pout.tile([128, FDH], f32, name="ot", tag="ot")
                    for jc in range(NJ):
                        c0 = 512 * jc
                        bclp = psb.tile([128, 512], f32, name="bclp", tag="bclp")
                        nc.tensor.matmul(
                            out=bclp[:],
                            lhsT=selm[0:16, 128 * h : 128 * h + 128],
                            rhs=liferow[0:16, c0 : c0 + 512],
                            start=True,
                            stop=True,
                            tile_position=(0, 0),
                        )
                        bcgp = psb.tile([128, 512], f32, name="bcgp", tag="bcgp")
                        nc.tensor.matmul(
                            out=bcgp[:],
                            lhsT=selm[0:16, 128 * h : 128 * h + 128],
                            rhs=gmrow[0:16, c0 : c0 + 512],
                            start=True,
                            stop=True,
                            tile_position=(0, 0),
                        )
                        t1 = pt1.tile([128, 512], bf16, name="t1", tag="t1", bufs=4)
                        nc.vector.tensor_mul(
                            out=t1[:], in0=dxt[:, c0 : c0 + 512], in1=bcgp[:]
                        )
                        nc.vector.tensor_mul(
                            out=out_t[:, c0 : c0 + 512],
                            in0=xt[:, HALO + c0 : HALO + c0 + 512],
                            in1=bclp[:],
                        )
                        nc.vector.tensor_add(
                            out=out_t[:, c0 : c0 + 512],
                            in0=out_t[:, c0 : c0 + 512],
                            in1=t1[:],
                        )
                    nc.sync.dma_start(
                        out=_ap(
                            out_f, ibase + HALFPX * h,
                            [[FDH, 8], [PLANE, 16], [1, FDH]],
                        ),
                        in_=out_t[:],
                    )
    return nc


def self_pool_max(nc, ppl, src, name):
    """3x3 wrap max-pool of a [128, 1026] pool-layout alpha tile.

    Returns [128, 512] tile of pooled values for the valid 512 px.
    Pool layout: partition p covers px [512p - 257, 512p + 769); flat index,
    image col j = (col - 1) mod 256.
    """
    mh = ppl.tile([128, 1026], f32, name=f"mh_{name}", tag="mh", bufs=1)
    # horizontal 3-max on [1, 1025)
    nc.vector.tensor_max(out=mh[:, 1:1025], in0=src[:, 0:1024], in1=src[:, 1:1025])
    nc.vector.tensor_max(out=mh[:, 1:1025], in0=mh[:, 1:1025], in1=src[:, 2:1026])
    # fix j=0 cols {1, 257, 513, 769}: max(src[c], src[c+1], src[c+255])
    nc.vector.tensor_max(
        out=_scols(mh, 1, 256, 4), in0=_scols(src, 1, 256, 4), in1=_scols(src, 2, 256, 4)
    )
    nc.vector.tensor_max(
        out=_scols(mh, 1, 256, 4), in0=_scols(mh, 1, 256, 4), in1=_scols(src, 256, 256, 4)
    )
    # fix j=255 cols {256, 512, 768}: max(src[c-1], src[c], src[c-255])
    nc.vector.tensor_max(
        out=_scols(mh, 256, 256, 3),
        in0=_scols(src, 255, 256, 3),
        in1=_scols(src, 256, 256, 3),
    )
    nc.vector.tensor_max(
        out=_scols(mh, 256, 256, 3),
        in0=_scols(mh, 256, 256, 3),
        in1=_scols(src, 1, 256, 3),
    )
    # vertical 3-max -> valid [257, 769)
    out = ppl.tile([128, 512], f32, name=f"pool_{name}", tag=f"po_{name}", bufs=1)
    nc.vector.tensor_max(out=out[:], in0=mh[:, 1:513], in1=mh[:, 257:769])
    nc.vector.tensor_max(out=out[:], in0=out[:], in1=mh[:, 513:1025])
    return out


def _host_weights(w1, b1, w2, b2):
    w1 = np.asarray(w1, np.float32)
    w2 = np.asarray(w2, np.float32)
    b1 = np.asarray(b1, np.float32)
    b2 = np.asarray(b2, np.float32)
    W1k = [w1[:, 0::3], w1[:, 1::3] * 0.125, w1[:, 2::3] * 0.125]
    wm1 = np.zeros((128, 768), np.float32)
    for s in range(4):
        for k in range(3):
            blkA = 128 * (2 * k)
            blkB = 128 * (2 * k + 1)
            wm1[32 * s : 32 * s + 16, blkA : blkA + 128] = W1k[k].T
            wm1[32 * s + 16 : 32 * s + 32, blkB : blkB + 128] = W1k[k].T
    wm2 = np.zeros((128, 48), np.float32)
    wm2[:, 0:16] = w2.T
    wm2[:, 32:48] = w2.T
    b1t = b1.reshape(128, 1)
    b2t = np.tile(b2 * STEP_SIZE, 8).reshape(128, 1)
    selm = np.zeros((16, 256), np.float32)
    for hh in range(2):
        for q in range(8):
            for c in range(16):
                selm[8 * hh + q, 128 * hh + 16 * q + c] = 1.0
    return (
        wm1.astype(ml_dtypes.bfloat16),
        wm2.astype(ml_dtypes.bfloat16),
        b1t.astype(np.float32),
        b2t.astype(np.float32),
        selm.astype(ml_dtypes.bfloat16),
    )


_NC_CACHE = {}


def _get_nc(n_img):
    if n_img not in _NC_CACHE:
        nc = bacc.Bacc("TRN2", target_bir_lowering=False, debug=False)
        build_kernel(nc, n_img)
        nc.compile()
        _NC_CACHE[n_img] = nc
    return _NC_CACHE[n_img]


def kernel(x, w1, b1, w2, b2, fire_mask):
    from concourse.bass_utils import run_bass_kernel_spmd

    x = np.asarray(x, np.float32).reshape(B, C, PLANE)
    fire = np.asarray(fire_mask, np.int32).reshape(B, PLANE)
    wm1, wm2, b1t, b2t, selm = _host_weights(w1, b1, w2, b2)

    nc = _get_nc(NIMG)
    in_maps = []
    for core in range(NCORE):
        sl = slice(core * NIMG, (core + 1) * NIMG)
        in_maps.append(
            {
                "xin": np.ascontiguousarray(x[sl]),
                "fire": np.ascontiguousarray(fire[sl]),
                "wm1": wm1,
                "wm2": wm2,
                "b1t": b1t,
                "b2t": b2t,
                "selm": selm,
            }
        )
    res = run_bass_kernel_spmd(nc, in_maps, core_ids=list(range(NCORE)))
    outs = [res.results[c]["out"].reshape(NIMG, C, H, Wd) for c in range(NCORE)]
    return np.concatenate(outs, axis=0)
